# revision 6
# baseline (speedup 1.0000x reference)
"""Trainium2 Bass kernel for nn_BaseEmbedLoss (segment-center cosine embedding loss).

Strategy (data-parallel over batch, 1 batch image per core x 8 cores):
  Pass 1 (on device, single pass over the 256MiB input):
    per 128-pixel group g: matmul  out += [feats|1|pad]_g^T @ [onehot_g | onehot_g*rinv_g]
    accumulated in PSUM across all groups -> [34, 40] with
      rows 0..31 = sums.T / nsum.T, row 32 = counts, cols split OH|OH*rinv.
    rinv = 1/||feat_pixel|| computed via ACT square + DVE reduce + sqrt + reciprocal.
  AllReduce (5KB) across the 8 cores.
  Tiny C x C center-similarity stage computed redundantly on every core.

Key identity: seg_cos[c] = centers[c] . nsum[c] / cnorm[c], nsum[c] = sum_{n in c} f_n/|f_n|
so no second pass over the data is needed.
"""

import sys

sys.path.insert(0, "/opt/trn_rl_repo")

import numpy as np
import ml_dtypes

import concourse.bass as bass
import concourse.mybir as mybir
import concourse.bacc as bacc
import concourse.tile as tile
from concourse import bass_utils

F32 = mybir.dt.float32
BF16 = mybir.dt.bfloat16
AF = mybir.ActivationFunctionType
ALU = mybir.AluOpType
AX = mybir.AxisListType

# Problem shapes (hardcoded per contract)
B, D, H, W = 8, 32, 512, 512
C = 19
CP = 20          # classes padded to even width (class 19 is a dummy)
NCORES = 8
HWL = H * W      # 262144 pixels per core (batch-sharded)
PX = 128         # pixels per matmul group (partition/contraction dim)
G = 64           # groups per supertile
FREE = G * D     # 2048 fp32 elements per partition per supertile
ST = int(__import__("os").environ.get("K_ST", HWL // (PX * G)))  # 32 supertiles
M = D + 2        # stationary cols: 32 dims + ones col + pad col


def _kernel_body(nc, tc, feats, labels, iota_c, ident, ident20, eye19, onesc, out_d):
    with (
        tc.tile_pool(name="consts", bufs=1) as cpool,
        tc.tile_pool(name="fio", bufs=3) as fpool,
        tc.tile_pool(name="work", bufs=2) as wpool,
        tc.tile_pool(name="small", bufs=3) as spool,
        tc.tile_pool(name="fin", bufs=1) as finpool,
        tc.tile_pool(name="accps", bufs=1, space="PSUM") as acc_pool,
        tc.tile_pool(name="ps", bufs=1, space="PSUM") as ps_pool,
        tc.tile_pool(name="dram", bufs=1, space="DRAM") as dpool,
    ):
        # ---- constants ----
        iota_sb = cpool.tile([PX, CP * G], BF16)
        nc.sync.dma_start(iota_sb[:], iota_c[:])
        ident_sb = cpool.tile([M, M], F32)
        nc.sync.dma_start(ident_sb[:], ident[:])
        ident20_sb = cpool.tile([CP, CP], F32)
        nc.sync.dma_start(ident20_sb[:], ident20[:])
        eye_sb = cpool.tile([CP, CP], F32)
        nc.sync.dma_start(eye_sb[:], eye19[:])
        ones_sb = cpool.tile([CP, 1], F32)
        nc.sync.dma_start(ones_sb[:], onesc[:])

        iota3 = iota_sb[:].rearrange("p (c g) -> p c g", c=CP)

        # persistent PSUM accumulator [M, 2*CP]
        acc = acc_pool.tile([M, 2 * CP], F32)
        acc3 = acc[:].rearrange("m (b c) -> m b c", b=2)

        # ---- main single pass ----
        for st in range(ST):
            F = fpool.tile([PX, FREE], F32, tag="F")
            nc.sync.dma_start(F[:], feats[st])
            L = fpool.tile([PX, G], BF16, tag="L")
            nc.sync.dma_start(L[:], labels[st])

            # per-pixel 1/||f||
            SQ = wpool.tile([PX, FREE], F32, tag="SQ")
            nc.scalar.square(SQ[:], F[:])
            nrm2 = spool.tile([PX, G], F32, tag="nrm2")
            nc.vector.reduce_sum(
                nrm2[:], SQ[:].rearrange("p (g d) -> p g d", g=G), axis=AX.X
            )
            nrm = spool.tile([PX, G], F32, tag="nrm")
            nc.scalar.sqrt(nrm[:], nrm2[:])
            rinv = spool.tile([PX, G], F32, tag="rinv")
            nc.vector.reciprocal(rinv[:], nrm[:])
            rinvb = spool.tile([PX, G], BF16, tag="rinvb")
            nc.vector.tensor_copy(rinvb[:], rinv[:])

            # stationary operand: bf16 [feats | 1 | pad] per group (gpsimd cast)
            Fb = wpool.tile([PX, G * M], BF16, tag="Fb")
            Fb3 = Fb[:].rearrange("p (g m) -> p g m", g=G)
            nc.gpsimd.memset(Fb3[:, :, D:M], 1.0)
            nc.gpsimd.tensor_copy(
                Fb3[:, :, 0:D], F[:].rearrange("p (g d) -> p g d", g=G)
            )

            # moving operand: [onehot | onehot*rinv], layout [p, b, c, g]
            Wt = wpool.tile([PX, 2 * CP * G], BF16, tag="Wt")
            W4 = Wt[:].rearrange("p (b c g) -> p b c g", b=2, c=CP)
            L_bc = L[:].rearrange("p (o g) -> p o g", o=1).broadcast_to([PX, CP, G])
            nc.vector.tensor_tensor(W4[:, 0], iota3, L_bc, op=ALU.is_equal)
            r_bc = rinvb[:].rearrange("p (o g) -> p o g", o=1).broadcast_to(
                [PX, CP, G]
            )
            nc.vector.tensor_tensor(W4[:, 1], W4[:, 0], r_bc, op=ALU.mult)

            for g in range(G):
                nc.tensor.matmul(
                    acc3,
                    Fb3[:, g, :],
                    W4[:, :, :, g],
                    start=(st == 0 and g == 0),
                    stop=(st == ST - 1 and g == G - 1),
                )

        # ---- all-reduce the [M, 2*CP] accumulator ----
        acc_sb = finpool.tile([M, 2 * CP], F32)
        nc.vector.tensor_copy(acc_sb[:], acc[:])
        cc_in = dpool.tile([M, 2 * CP], F32)
        cc_out = dpool.tile([M, 2 * CP], F32)
        nc.gpsimd.dma_start(cc_in[:], acc_sb[:])
        nc.gpsimd.collective_compute(
            "AllReduce",
            ALU.add,
            replica_groups=[list(range(NCORES))],
            ins=[cc_in[:].opt()],
            outs=[cc_out[:].opt()],
        )
        ar_sb = finpool.tile([M, 2 * CP], F32)
        nc.gpsimd.dma_start(ar_sb[:], cc_out[:])

        # ---- transpose to class-major (each block separately so both land on
        # partitions 0..CP-1; DVE lanes can't cross partitions) ----
        tps = ps_pool.tile([CP, M], F32, tag="tps")
        nc.tensor.transpose(tps[:], ar_sb[:, 0:CP], ident_sb[:])
        TA = finpool.tile([CP, M], F32)
        nc.vector.tensor_copy(TA[:], tps[:])
        tps_b = ps_pool.tile([CP, M], F32, tag="tps_b")
        nc.tensor.transpose(tps_b[:], ar_sb[:, CP : 2 * CP], ident_sb[:])
        TBn = finpool.tile([CP, M], F32)
        nc.vector.tensor_copy(TBn[:], tps_b[:])

        counts = TA[0:CP, D : D + 1]
        sums = TA[0:CP, 0:D]
        nsum = TBn[0:CP, 0:D]

        def small(shape, tag, dt=F32):
            return finpool.tile(shape, dt, tag=tag, name=tag)

        denom = small([CP, 1], "denom")
        nc.vector.tensor_scalar_max(denom[:], counts, 1.0)
        rden = small([CP, 1], "rden")
        nc.vector.reciprocal(rden[:], denom[:])
        present = small([CP, 1], "present")
        nc.vector.tensor_scalar_min(present[:], counts, 1.0)

        centers = small([CP, D], "centers")
        nc.vector.tensor_scalar_mul(centers[:], sums, rden[:])

        csq = small([CP, D], "csq")
        cn2 = small([CP, 1], "cn2")
        nc.vector.tensor_mul(csq[:], centers[:], centers[:])
        nc.vector.reduce_sum(cn2[:], csq[:], axis=AX.X)
        cnorm = small([CP, 1], "cnorm")
        nc.scalar.sqrt(cnorm[:], cn2[:])
        cnc = small([CP, 1], "cnc")
        nc.vector.tensor_scalar_max(cnc[:], cnorm[:], 1e-30)
        rcn = small([CP, 1], "rcn")
        nc.vector.reciprocal(rcn[:], cnc[:])

        dotp = small([CP, D], "dotp")
        dotcn = small([CP, 1], "dotcn")
        nc.vector.tensor_mul(dotp[:], centers[:], nsum)
        nc.vector.reduce_sum(dotcn[:], dotp[:], axis=AX.X)
        mean_cos = small([CP, 1], "mean_cos")
        nc.vector.tensor_scalar(
            mean_cos[:], dotcn[:], rcn[:], rden[:], op0=ALU.mult, op1=ALU.mult
        )
        simc = small([CP, 1], "simc")
        nc.scalar.activation(simc[:], mean_cos[:], AF.Copy, bias=1.0, scale=-1.0)
        sim_contrib = small([CP, 1], "sim_contrib")
        nc.vector.tensor_mul(sim_contrib[:], simc[:], present[:])

        # cosM = (centers*rcn) @ (centers*rcn).T
        cs = small([CP, D], "cs")
        nc.vector.tensor_scalar_mul(cs[:], centers[:], rcn[:])
        tps2 = ps_pool.tile([D, CP], F32, tag="tps2")
        nc.tensor.transpose(tps2[:], cs[:], ident20_sb[:])
        cs_T = small([D, CP], "cs_T")
        nc.vector.tensor_copy(cs_T[:], tps2[:])
        cos_ps = ps_pool.tile([CP, CP], F32, tag="cos_ps")
        nc.tensor.matmul(cos_ps[:], cs_T[:], cs_T[:], start=True, stop=True)
        cosM = small([CP, CP], "cosM")
        nc.vector.tensor_copy(cosM[:], cos_ps[:])

        R = small([CP, CP], "R")
        nc.vector.tensor_relu(R[:], cosM[:])
        t1 = small([CP, CP], "t1")
        nc.scalar.activation(t1[:], cosM[:], AF.Copy, bias=1.0, scale=-1.0)
        A = small([CP, CP], "A")
        nc.vector.tensor_sub(A[:], t1[:], R[:])
        t2 = small([CP, CP], "t2")
        nc.vector.tensor_mul(t2[:], A[:], eye_sb[:])
        terms = small([CP, CP], "terms")
        nc.vector.tensor_add(terms[:], R[:], t2[:])
        rowsum = small([CP, 1], "rowsum")
        nc.vector.reduce_sum(rowsum[:], terms[:], axis=AX.X)
        diffc = small([CP, 1], "diffc")
        nc.scalar.mul(diffc[:], rowsum[:], 1.0 / C)
        diff_contrib = small([CP, 1], "diff_contrib")
        nc.vector.tensor_mul(diff_contrib[:], diffc[:], present[:])

        contrib = small([CP, 1], "contrib")
        nc.vector.tensor_add(contrib[:], sim_contrib[:], diff_contrib[:])
        fin_ps = ps_pool.tile([1, 1], F32, tag="fin_ps")
        nc.tensor.matmul(fin_ps[:], contrib[:], ones_sb[:], start=True, stop=True)
        fin_sb = small([1, 1], "fin_sb")
        nc.vector.tensor_copy(fin_sb[:], fin_ps[:])
        nc.sync.dma_start(out_d[:], fin_sb[:])


_CACHE = {}


def _build_nc():
    if "nc" in _CACHE:
        return _CACHE["nc"]
    nc = bacc.Bacc(
        "TRN2", target_bir_lowering=False, debug=False, num_devices=NCORES
    )
    feats = nc.dram_tensor("feats", [ST, PX, FREE], F32, kind="ExternalInput")
    labels = nc.dram_tensor("labels", [ST, PX, G], BF16, kind="ExternalInput")
    iota_c = nc.dram_tensor("iota_c", [PX, CP * G], BF16, kind="ExternalInput")
    ident = nc.dram_tensor("ident", [M, M], F32, kind="ExternalInput")
    ident20 = nc.dram_tensor("ident20", [CP, CP], F32, kind="ExternalInput")
    eye19 = nc.dram_tensor("eye19", [CP, CP], F32, kind="ExternalInput")
    onesc = nc.dram_tensor("onesc", [CP, 1], F32, kind="ExternalInput")
    out_d = nc.dram_tensor("out", [1, 1], F32, kind="ExternalOutput")
    with tile.TileContext(nc) as tc:
        _kernel_body(
            nc, tc, feats, labels, iota_c, ident, ident20, eye19, onesc, out_d
        )
    nc.compile()
    _CACHE["nc"] = nc
    return nc


def _consts():
    if "consts" in _CACHE:
        return _CACHE["consts"]
    iota = np.broadcast_to(
        np.arange(CP, dtype=np.float32).reshape(1, CP, 1), (PX, CP, G)
    )
    iota = np.ascontiguousarray(iota.reshape(PX, CP * G)).astype(ml_dtypes.bfloat16)
    ident = np.eye(M, dtype=np.float32)
    ident20 = np.eye(CP, dtype=np.float32)
    eye19 = np.eye(CP, dtype=np.float32)
    eye19[C, C] = 0.0  # dummy padded class contributes nothing
    onesc = np.ones((CP, 1), dtype=np.float32)
    _CACHE["consts"] = (iota, ident, ident20, eye19, onesc)
    return _CACHE["consts"]


def _shard_inputs(inputs, targets):
    """Host-side marshalling: batch-shard + retile to the DMA-friendly layout."""
    inputs = np.asarray(inputs, dtype=np.float32)
    targets = np.asarray(targets)
    iota, ident, ident20, eye19, onesc = _consts()
    in_maps = []
    for b in range(NCORES):
        # [D, H, W] -> [N, D] pixel-major (matches reference transpose/reshape)
        f = np.ascontiguousarray(inputs[b].transpose(1, 2, 0).reshape(HWL, D))
        # -> [ST, PX, G, D] -> [ST, PX, FREE]
        f = np.ascontiguousarray(
            f.reshape(ST, G, PX, D).transpose(0, 2, 1, 3)
        ).reshape(ST, PX, FREE)
        lab = targets[b].reshape(HWL).astype(np.float32)
        lab = np.ascontiguousarray(
            lab.reshape(ST, G, PX).transpose(0, 2, 1)
        ).astype(ml_dtypes.bfloat16)
        in_maps.append(
            {
                "feats": f,
                "labels": lab,
                "iota_c": iota,
                "ident": ident,
                "ident20": ident20,
                "eye19": eye19,
                "onesc": onesc,
            }
        )
    return in_maps


def run_on_device(in_maps):
    nc = _build_nc()
    res = bass_utils.run_bass_kernel_spmd(
        nc, in_maps, core_ids=list(range(NCORES))
    )
    return res


def kernel(inputs, targets, num_classes):
    assert int(num_classes) == C
    in_maps = _shard_inputs(inputs, targets)
    res = run_on_device(in_maps)
    out = np.asarray(res.results[0]["out"], dtype=np.float32).reshape(1)
    return out


if __name__ == "__main__":
    # smoke test with random data
    rng = np.random.default_rng(0)
    x = rng.standard_normal((B, D, H, W), dtype=np.float32)
    t = rng.integers(0, C, size=(B, H, W)).astype(np.int64)
    print(kernel(x, t, C))
